# revision 55
# baseline (speedup 1.0000x reference)
"""Trainium2 Bass kernel for nn_MultiHeadAttention_81363860455568.

Reference computation (B=2, S=2048, D=1024, H=16, DK=64):
    qh = split_heads(q @ Wq.T); kh, vh likewise
    scores = softmax(qh @ kh.T / 8, axis=-1)
    scores = scores * reaches[:,None,None,:]            (per key)
    scores = scores * (1 - 0.999999*eye(S))             (diagonal suppression)
    out = vh - scores @ vh
    out = out * contrib[:,None,:,None]                  (per query)
    y = concat_heads(out) @ Wo.T

Sharding: 8 cores = 2 batches x 4 head-groups (4 heads each). Each core gets
its batch's activations plus the head-group weight slices and returns a
partial y [S, D] (bf16) the host sums in fp32 across the 4 head-groups.

The cost model charges a matmul by its output free-dim size only (K and M
are free), ACT by free-dim columns, so the design packs the K/M dims and
keeps the exp stream (the true floor: 16.8M columns / 128 partitions at
0.83ns) saturated from ~9us onward:
  - Q/K projections in fp8 e4m3 with DoubleRow matmuls (K=256/instruction
    at 0.5 cycles/row) into bf16 head-pair tiles qhT2/khT2 [2h x 64d, S].
  - Scores per head: bf16 K=64 matmuls -> scoresT [128 k, 1024 q] PSUM.
  - exp on ACT (scale=1/8, bias=-2; softmax is shift-invariant, the shift
    adds fp8 headroom), output fp8 e5m2 into et tiles [128k, 2hl, 16kb, 1024q].
  - Diagonal suppression on the otherwise-idle Pool engine: affine_select
    copies the diag into d2 and zeroes it in-place in et.
  - AV in the "natural" orientation: av[128 q, 65] accumulates 8 fp8e5m2
    DoubleRow steps, lhsT=et [128,2,128], rhs=vnat [128,2,65] where vnat is
    reach-scaled V in [k, d] layout augmented with a ones column -- so
    av[:,64] is the softmax denominator for free. The diag contribution is
    added back with one N=1 matmul (lhsT=d2, rhs=ones).
  - Epilogue per (head, q-block): s1 = contrib/denom (DVE reciprocal+mul),
    then one fused scalar_tensor_tensor: catn = (av*s1) - vh*contrib.
    catn is the negated output, fixed up by negating Wo host-side.
  - catn [q, d] is PE-transposed to catT [d, q], then Wo (bf16) produces
    y[q-block, :], staged through SBUF (DVE in the body, ACT at the tail
    where the exp stream has ended) and DMA'd out as bf16 partials.
Scheduling: 4 B1 phases (q-half x head-pair) of scores+exp with V-projection
jobs, late projection groups and the previous phase's AV/epilogue/Wo work
interleaved into per-key-block PE slots; input DMAs are split so the first
exp fires as soon as the minimal K/Q bytes have landed.
V stays bf16 end-to-end (vh feeds the output directly); the fp8 paths only
perturb softmax weights, which the output is insensitive to.
"""

import functools

import numpy as np
import ml_dtypes

import concourse.bass as bass
import concourse.mybir as mybir
import concourse.tile as tile
from concourse import bacc
from concourse.bass_utils import run_bass_kernel_spmd
from concourse.masks import make_identity

BF16 = mybir.dt.bfloat16
F32 = mybir.dt.float32
F8 = mybir.dt.float8e4          # e4m3: Q/K inputs + weights (range +-240)
F8AV = mybir.dt.float8e5        # e5m2: exp'd scores + V for AV (wide range)
NP_F8 = ml_dtypes.float8_e4m3
NP_BF16 = ml_dtypes.bfloat16

B, S, D, H = 2, 2048, 1024, 16
DK = D // H          # 64
HG = 4               # heads per core (head group)
GD = HG * DK         # 256 head-group dims per core
NKC = D // 128       # 8 contraction chunks for projections
NKB = S // 128       # 16 key blocks
EXPB = -2.0          # exp bias; softmax-invariant, keeps e^x in fp8 range

DR = mybir.MatmulPerfMode.DoubleRow
Exp = mybir.ActivationFunctionType.Exp
Copy = mybir.ActivationFunctionType.Copy
MUL = mybir.AluOpType.mult
SUB = mybir.AluOpType.subtract
EQ = mybir.AluOpType.is_equal
NE = mybir.AluOpType.not_equal


def _emit_kernel(tc: tile.TileContext):
    nc = tc.nc

    # DRAM params. Host pre-reshapes so each input is one large DMA with
    # 2KB+ per-partition contiguous runs.
    qT8 = nc.declare_dram_parameter("qT8", [128, NKC, S], F8, isOutput=False).ap()
    kT8 = nc.declare_dram_parameter("kT8", [128, NKC, S], F8, isOutput=False).ap()
    vT = nc.declare_dram_parameter("vT", [128, NKC, S], BF16, isOutput=False).ap()
    wq8 = nc.declare_dram_parameter("wq8", [128, 2, NKC, 128], F8, isOutput=False).ap()
    wk8 = nc.declare_dram_parameter("wk8", [128, 2, NKC, 128], F8, isOutput=False).ap()
    wv = nc.declare_dram_parameter("wv", [128, NKC, GD], BF16, isOutput=False).ap()
    wo = nc.declare_dram_parameter("wo", [128, 2, D], BF16, isOutput=False).ap()
    rcol = nc.declare_dram_parameter("rcol", [128, NKB], F32, isOutput=False).ap()
    ccol = nc.declare_dram_parameter("ccol", [128, NKB], F32, isOutput=False).ap()
    y = nc.declare_dram_parameter("y", [S, D], BF16, isOutput=True).ap()

    # ---------------- resident SBUF ----------------
    consts = tc.alloc_tile_pool(name="consts", bufs=1)
    wq_sb = consts.tile([128, 2, NKC, 128], F8)
    wk_sb = consts.tile([128, 2, NKC, 128], F8)
    wv_sb = consts.tile([128, NKC, GD], BF16)
    wo_sb = consts.tile([128, 2, D], BF16)
    rr = consts.tile([128, NKB], F32)
    cc = consts.tile([128, NKB], F32)
    ones1 = consts.tile([128, 1], F8AV)
    ident = consts.tile([128, 128], BF16)
    actwarm = consts.tile([1, 2], F32)
    expbias = consts.tile([128, 1], F32)

    res = tc.alloc_tile_pool(name="res", bufs=1)
    # projected Q/K, bf16, head-pair layout: partitions = [head 2p | head 2p+1]
    qhT2 = [res.tile([128, S], BF16, name=f"qhT2_{p}") for p in range(2)]
    khT2 = [res.tile([128, S], BF16, name=f"khT2_{p}") for p in range(2)]
    vhc = res.tile([128, NKB, GD], BF16)     # vh * contrib, natural [s, d]
    vnat = res.tile([128, NKB, 264], F8AV)     # reaches-scaled V + ones cols
    catT = res.tile([128, 2, S], BF16)       # [d-half, q] transposed output
    catn = res.tile([128, 8, GD], BF16)      # negated epilogue out, natural
    # et[par]: exp'd scores for one phase (2 heads x 16 kb x 1024 q), fp8
    et = [res.tile([128, 2, NKB, 1024], F8AV, name=f"et_{p}") for p in range(2)]
    qT_sb = res.tile([128, NKC, S], F8)
    kT_sb = res.tile([128, NKC, S], F8)
    vT_sb = res.tile([128, NKC, S], BF16)
    consts.seal()
    res.seal()

    # constants
    nc.gpsimd.memset(ones1, 1.0)
    nc.gpsimd.memset(actwarm, 0.0)
    nc.gpsimd.memset(expbias, EXPB)
    make_identity(nc, ident)
    for h in range(HG):
        nc.gpsimd.memset(vnat[:, :, h * 66 + 64:h * 66 + 65], 1.0)
        nc.gpsimd.memset(vnat[:, :, h * 66 + 65:h * 66 + 66], 0.0)

    # input DMAs split and ordered so the first scores+exp can fire as early
    # as possible: the kT quarter covering key blocks 0-3, pair-0 weights,
    # the two qT quarters feeding q-half 0, then the rest in need order.
    def dmas(dst, src, lo, hi):
        nc.sync.dma_start(out=dst[:, :, lo:hi], in_=src[:, :, lo:hi])

    dmas(kT_sb, kT8, 0, 512)
    nc.sync.dma_start(out=wk_sb[:, 0], in_=wk8[:, 0])
    nc.sync.dma_start(out=wq_sb[:, 0], in_=wq8[:, 0])
    dmas(qT_sb, qT8, 0, 512)
    dmas(qT_sb, qT8, 512, 1024)
    nc.sync.dma_start(out=wk_sb[:, 1], in_=wk8[:, 1])
    nc.sync.dma_start(out=wq_sb[:, 1], in_=wq8[:, 1])
    dmas(kT_sb, kT8, 512, 1024)
    dmas(kT_sb, kT8, 1024, 2048)
    nc.sync.dma_start(out=wv_sb, in_=wv)
    nc.sync.dma_start(out=rr, in_=rcol)
    nc.sync.dma_start(out=cc, in_=ccol)
    dmas(vT_sb, vT, 0, 1024)
    dmas(vT_sb, vT, 1024, 2048)
    dmas(qT_sb, qT8, 1024, 2048)
    nc.sync.dma_start(out=wo_sb, in_=wo)

    with (
        tc.tile_pool(name="sc", bufs=2, space="PSUM") as sc,
        tc.tile_pool(name="acc", bufs=2, space="PSUM") as acc,
        tc.tile_pool(name="two", bufs=1, space="PSUM") as two,
        tc.tile_pool(name="dpool", bufs=34) as dpool,
        tc.tile_pool(name="spool", bufs=8) as spool,
        tc.tile_pool(name="ypool", bufs=6) as ypool,
    ):
        # Preload the exp activation table off the critical path.
        wrm = spool.tile([1, 2], F32, tag="wrm")
        nc.scalar.activation(wrm, actwarm, Exp)

        # Warm the PE p-state while input DMAs are in flight: ~4us of dummy
        # transposes (no DMA deps) so the first projections run at full rate.
        for _ in range(5):
            wtp = acc.tile([128, 512], F32, tag="acc", name="wtp")
            for j in range(4):
                nc.tensor.transpose(
                    wtp[:, j * 128:(j + 1) * 128].bitcast(BF16)[:, 0:128],
                    ident, ident)

        # ---- Q/K projections: fp8 DoubleRow into bf16 head-pair tiles ----
        def proj_grp(w_sb, x_sb, dst, p, lo, hi):
            ps = acc.tile([128, 512], F32, tag="acc", name="ps")
            for j in range(4):
                nc.tensor.matmul(
                    ps[:, 0:hi - lo],
                    lhsT=w_sb[:, p, 2 * j:2 * j + 2, :],
                    rhs=x_sb[:, 2 * j:2 * j + 2, lo:hi],
                    start=(j == 0), stop=(j == 3),
                    perf_mode=DR,
                )
            nc.vector.tensor_copy(dst[p][:, lo:hi], ps[:, 0:hi - lo])

        def kgrp(p, nq):
            proj_grp(wk_sb, kT_sb, khT2, p, nq * 512, (nq + 1) * 512)

        def qgrp(p, nq):
            proj_grp(wq_sb, qT_sb, qhT2, p, nq * 512, (nq + 1) * 512)

        # critical lead-in: only what B1 phase 0's first key blocks and
        # q-half-0 need, in DMA-arrival order
        kgrp(0, 0)
        qgrp(0, 0)
        qgrp(0, 1)

        # ---- V projection job for one 128-row block of S ----
        def vjob(ms):
            ps = acc.tile([128, 256], F32, tag="acc", name="ps")
            for kc in range(NKC):
                nc.tensor.matmul(
                    ps,
                    lhsT=vT_sb[:, kc, ms * 128:(ms + 1) * 128],
                    rhs=wv_sb[:, kc, :],
                    start=(kc == 0), stop=(kc == NKC - 1),
                )
            nc.vector.tensor_scalar_mul(vhc[:, ms, :], ps, cc[:, ms:ms + 1])
            for h in range(HG):
                nc.vector.tensor_scalar_mul(
                    vnat[:, ms, h * 66:h * 66 + 64],
                    ps[:, h * 64:(h + 1) * 64], rr[:, ms:ms + 1])

        d2map = {}

        # ---- B2 work for one (consumed phase, global q-block) ----
        def b2qb(ph, qb, tail=False):
            half, hp = divmod(ph, 2)
            par = hp
            qoff = (qb % 8) * 128
            for hl in range(2):
                h = hp * 2 + hl
                av = acc.tile([128, 256], F32, tag="acc", name="av")
                for t in range(8):
                    nc.tensor.matmul(
                        av[:, 0:65],
                        lhsT=et[par][:, hl, 2 * t:2 * t + 2, qoff:qoff + 128],
                        rhs=vnat[:, 2 * t:2 * t + 2, h * 66:h * 66 + 65],
                        start=(t == 0), stop=(t == 7),
                        perf_mode=DR,
                    )
                    if t == 0:
                        # diag add-back: denominator must see the unmasked sum
                        nc.tensor.matmul(
                            av[:, 64:65],
                            lhsT=d2map[(ph, hl, qb)],
                            rhs=ones1,
                            start=False, stop=False,
                            skip_group_check=True,
                        )
                rec = spool.tile([128, 1], F32, tag="rec")
                nc.vector.reciprocal(rec, av[:, 64:65])
                s1 = spool.tile([128, 1], F32, tag="s1")
                nc.vector.tensor_mul(s1, rec, cc[:, qb:qb + 1])
                nc.vector.scalar_tensor_tensor(
                    out=catn[:, qb % 8, h * 64:(h + 1) * 64],
                    in0=av[:, 0:64],
                    scalar=s1,
                    in1=vhc[:, qb, h * 64:(h + 1) * 64],
                    op0=MUL, op1=SUB,
                )
            if hp == 1:
                if tail:
                    # Body-Wo's "two" slot (2 banks) is idle at tail: one
                    # transpose per bank, then a single merged DVE copy.
                    tp = two.tile([128, 2, 1024], BF16, tag="two", name="tp")
                    for dh in range(2):
                        nc.tensor.transpose(
                            tp[:, dh, 0:128],
                            catn[:, qb % 8, dh * 128:(dh + 1) * 128], ident)
                    nc.vector.tensor_copy(
                        catT[:, :, qb * 128:(qb + 1) * 128], tp[:, :, 0:128])
                else:
                    for dh in range(2):
                        tp = acc.tile([128, 128], BF16, tag="acc", name="tp")
                        nc.tensor.transpose(
                            tp, catn[:, qb % 8, dh * 128:(dh + 1) * 128], ident)
                        nc.vector.tensor_copy(
                            catT[:, dh, qb * 128:(qb + 1) * 128], tp)
                # At tail the scores pool is idle; borrowing its (same-sized)
                # slots gives Wo a 2-deep ring without extra PSUM banks.
                pool = sc if tail else two
                wop = pool.tile([128, 2, 512], F32, tag="sc" if tail else "two",
                                name="wop")
                for oc in range(2):
                    for dh in range(2):
                        nc.tensor.matmul(
                            wop[:, oc, :],
                            lhsT=catT[:, dh, qb * 128:(qb + 1) * 128],
                            rhs=wo_sb[:, dh, oc * 512:(oc + 1) * 512],
                            start=(dh == 0), stop=(dh == 1),
                        )
                ysb = ypool.tile([128, 1024], BF16, tag="ysb")
                if tail:
                    # per-oc copy + DMA: the copy starts right after its own
                    # oc's matmuls, and the last drain unit is smaller
                    for oc in range(2):
                        eng = nc.scalar if (qb % 2 == oc) else None
                        sl = ysb[:, oc * 512:(oc + 1) * 512]
                        if qb >= 14 and oc == 1:
                            nc.vector.tensor_copy(sl, wop[:, oc, :])
                        else:
                            nc.scalar.activation(sl, wop[:, oc, :], Copy)
                        nc.sync.dma_start(
                            out=y[qb * 128:(qb + 1) * 128,
                                  oc * 512:(oc + 1) * 512],
                            in_=sl)
                else:
                    nc.vector.tensor_copy(ysb, wop)
                    nc.sync.dma_start(
                        out=y[qb * 128:(qb + 1) * 128, :], in_=ysb)

        # ---- B1 phase: scores + exp (+ diag) with interleaved fill jobs ----
        # Per-phase PE fill schedules, keyed by kb slot. Late projection
        # groups and V jobs ride in phase 0/1 slots (ordered so their DMA
        # halves have landed); B2 consumption starts in phase 1's second
        # half (vnat complete by then) and runs on odd slots afterwards.
        def sched_jobs(i, kb):
            jobs = []
            if i == 0:
                if kb == 0:
                    jobs.append(lambda: kgrp(0, 1))
                elif kb == 1:
                    jobs.append(lambda: kgrp(0, 2))
                elif kb == 2:
                    jobs.append(lambda: kgrp(0, 3))
                elif kb == 4:
                    jobs += [lambda: kgrp(1, 0), lambda: kgrp(1, 1)]
                elif kb == 5:
                    jobs += [lambda: kgrp(1, 2), lambda: kgrp(1, 3)]
                elif kb == 6:
                    jobs += [lambda: qgrp(1, 0), lambda: qgrp(1, 1)]
                elif kb >= 8:
                    ms = kb - 8
                    jobs.append(lambda: vjob(ms))
            elif i == 1:
                if kb < 8:
                    ms = kb + 8
                    jobs.append(lambda: vjob(ms))
                else:
                    jobs.append(lambda: b2qb(0, kb - 8))
                    if kb % 2 == 0:
                        # late q projections ride the PE-light B2 slots
                        pj, pn = divmod((kb - 8) // 2, 2)
                        jobs.append(lambda: qgrp(pj, 2 + pn))
            else:
                if kb % 2 == 1:
                    prev = i - 1
                    qb = 8 * (prev // 2) + kb // 2
                    jobs.append(lambda: b2qb(prev, qb))
            return jobs

        def b1phase(i):
            half, hp = divmod(i, 2)
            par = hp
            q0 = half * 1024
            if i == 0:
                # Bootstrap kb0 at qc granularity: both heads' first-half
                # exps only need the first projected q chunk, so they fill
                # the wait for the second chunk's DMA+projection.
                sps = [sc.tile([128, 1024], F32, tag="sc", name="sp")
                       for _ in range(2)]
                for qc in range(2):
                    for hl in range(2):
                        r0, r1 = hl * 64, hl * 64 + 64
                        nc.tensor.matmul(
                            sps[hl][:, qc * 512:(qc + 1) * 512],
                            lhsT=khT2[0][r0:r1, 0:128],
                            rhs=qhT2[0][r0:r1, qc * 512:(qc + 1) * 512],
                            start=True, stop=True,
                        )
                        nc.scalar.activation(
                            et[0][:, hl, 0, qc * 512:(qc + 1) * 512],
                            sps[hl][:, qc * 512:(qc + 1) * 512],
                            Exp, scale=0.125, bias=expbias)
                    if qc == 0:
                        for hl in range(2):
                            esl = et[0][:, hl, 0, 0:128]
                            d2t = dpool.tile([128, 128], F8AV, tag="d2",
                                             name="d2t")
                            nc.gpsimd.affine_select(
                                out=d2t, in_=esl, compare_op=EQ,
                                fill=0.0, base=0, pattern=[[-1, 128]],
                                channel_multiplier=1)
                            nc.gpsimd.affine_select(
                                out=esl, in_=esl, compare_op=NE,
                                fill=0.0, base=0, pattern=[[-1, 128]],
                                channel_multiplier=1)
                            d2map[(0, hl, 0)] = d2t
            for kb in range(NKB):
                for hl in range(2):
                    if i == 0 and kb == 0:
                        continue
                    r0, r1 = hl * 64, hl * 64 + 64
                    sp = sc.tile([128, 1024], F32, tag="sc", name="sp")
                    for qc in range(2):
                        nc.tensor.matmul(
                            sp[:, qc * 512:(qc + 1) * 512],
                            lhsT=khT2[hp][r0:r1, kb * 128:(kb + 1) * 128],
                            rhs=qhT2[hp][r0:r1, q0 + qc * 512:q0 + (qc + 1) * 512],
                            start=True, stop=True,
                        )
                    nc.scalar.activation(
                        et[par][:, hl, kb, :], sp, Exp, scale=0.125,
                        bias=expbias)
                    if 8 * half <= kb < 8 * half + 8:
                        off = (kb - 8 * half) * 128
                        esl = et[par][:, hl, kb, off:off + 128]
                        d2t = dpool.tile([128, 128], F8AV, tag="d2", name="d2t")
                        nc.gpsimd.affine_select(
                            out=d2t, in_=esl, compare_op=EQ,
                            fill=0.0, base=0, pattern=[[-1, 128]],
                            channel_multiplier=1)
                        nc.gpsimd.affine_select(
                            out=esl, in_=esl, compare_op=NE,
                            fill=0.0, base=0, pattern=[[-1, 128]],
                            channel_multiplier=1)
                        d2map[(i, hl, kb)] = d2t
                for job in sched_jobs(i, kb):
                    job()

        for i in range(4):
            b1phase(i)
        for qb in range(8, 16):
            b2qb(3, qb, tail=True)


@functools.cache
def build_nc() -> bass.Bass:
    nc = bacc.Bacc("TRN2", target_bir_lowering=False, debug=False)
    with tile.TileContext(nc) as tc:
        _emit_kernel(tc)
    nc.compile()
    return nc


def _to_chunked(x, dtype, chunk=128):
    """[R, C] -> [chunk, R//chunk, C] so partition p's data is contiguous."""
    r, c = x.shape
    return np.ascontiguousarray(
        x.reshape(r // chunk, chunk, c).transpose(1, 0, 2)).astype(dtype)


def _to_pairmajor(w, dtype):
    """[D, 256] -> [128, 2, 8, 128]: per-partition pair-major contiguous."""
    c = _to_chunked(w, dtype)              # [128, 8, 256]
    return np.ascontiguousarray(
        c.reshape(128, 8, 2, 128).transpose(0, 2, 1, 3))


def _prep_inputs(q, k, v, reaches, Wq, Wk, Wv, Wo):
    """Host-side shard + layout prep. Returns per-core input maps."""
    r = np.asarray(reaches, np.float32)
    rs = r.sum(axis=-1, keepdims=True)
    contrib = (rs - r) / (rs + 1e-9) * (1.0 - r) * 100.0  # [B, S] f32

    per_batch = []
    for b in range(B):
        qTb = _to_chunked(np.asarray(q[b], np.float32).T, NP_F8)
        kTb = _to_chunked(np.asarray(k[b], np.float32).T, NP_F8)
        vTb = _to_chunked(np.asarray(v[b], np.float32).T, NP_BF16)
        rcol = np.ascontiguousarray(r[b].reshape(NKB, 128).T)
        ccol = np.ascontiguousarray(contrib[b].reshape(NKB, 128).T)
        per_batch.append((qTb, kTb, vTb, rcol, ccol))

    in_maps = []
    for c in range(8):
        b, g = divmod(c, 4)
        hs = slice(g * GD, (g + 1) * GD)
        qTb, kTb, vTb, rcol, ccol = per_batch[b]
        wqh = np.asarray(Wq, np.float32)[hs, :].T
        wkh = np.asarray(Wk, np.float32)[hs, :].T
        wvh = np.asarray(Wv, np.float32)[hs, :].T
        woh = -np.asarray(Wo, np.float32)[:, hs].T
        in_maps.append({
            "qT8": qTb, "kT8": kTb, "vT": vTb,
            "wq8": _to_pairmajor(wqh, NP_F8),
            "wk8": _to_pairmajor(wkh, NP_F8),
            "wv": _to_chunked(wvh, NP_BF16),
            "wo": _to_chunked(woh, NP_BF16),
            "rcol": rcol, "ccol": ccol,
        })
    return in_maps


def kernel(q, k, v, reaches, Wq, Wk, Wv, Wo, **run_kwargs):
    nc = build_nc()
    in_maps = _prep_inputs(q, k, v, reaches, Wq, Wk, Wv, Wo)
    res = run_bass_kernel_spmd(nc, in_maps, list(range(8)), **run_kwargs)
    out = np.zeros((B, S, D), np.float32)
    for c in range(8):
        b = c // 4
        out[b] += np.asarray(res.results[c]["y"], np.float32)
    if run_kwargs:
        kernel.last_results = res
    return out


# revision 56
# speedup vs baseline: 1.0108x; 1.0108x over previous
"""Trainium2 Bass kernel for nn_MultiHeadAttention_81363860455568.

Reference computation (B=2, S=2048, D=1024, H=16, DK=64):
    qh = split_heads(q @ Wq.T); kh, vh likewise
    scores = softmax(qh @ kh.T / 8, axis=-1)
    scores = scores * reaches[:,None,None,:]            (per key)
    scores = scores * (1 - 0.999999*eye(S))             (diagonal suppression)
    out = vh - scores @ vh
    out = out * contrib[:,None,:,None]                  (per query)
    y = concat_heads(out) @ Wo.T

Sharding: 8 cores = 2 batches x 4 head-groups (4 heads each). Each core gets
its batch's activations plus the head-group weight slices and returns a
partial y [S, D] (bf16) the host sums in fp32 across the 4 head-groups.

The cost model charges a matmul by its output free-dim size only (K and M
are free), ACT by free-dim columns, so the design packs the K/M dims and
keeps the exp stream (the true floor: 16.8M columns / 128 partitions at
0.83ns) saturated from ~9us onward:
  - Q/K projections in fp8 e4m3 with DoubleRow matmuls (K=256/instruction
    at 0.5 cycles/row) into bf16 head-pair tiles qhT2/khT2 [2h x 64d, S].
  - Scores per head: bf16 K=64 matmuls -> scoresT [128 k, 1024 q] PSUM.
  - exp on ACT (scale=1/8, bias=-2; softmax is shift-invariant, the shift
    adds fp8 headroom), output fp8 e5m2 into et tiles [128k, 2hl, 16kb, 1024q].
  - Diagonal suppression on the otherwise-idle Pool engine: affine_select
    copies the diag into d2 and zeroes it in-place in et.
  - AV in the "natural" orientation: av[128 q, 65] accumulates 8 fp8e5m2
    DoubleRow steps, lhsT=et [128,2,128], rhs=vnat [128,2,65] where vnat is
    reach-scaled V in [k, d] layout augmented with a ones column -- so
    av[:,64] is the softmax denominator for free. The diag contribution is
    added back with one N=1 matmul (lhsT=d2, rhs=ones).
  - Epilogue per (head, q-block): s1 = contrib/denom (DVE reciprocal+mul),
    then one fused scalar_tensor_tensor: catn = (av*s1) - vh*contrib.
    catn is the negated output, fixed up by negating Wo host-side.
  - catn [q, d] is PE-transposed to catT [d, q], then Wo (bf16) produces
    y[q-block, :], staged through SBUF (DVE in the body, ACT at the tail
    where the exp stream has ended) and DMA'd out as bf16 partials.
Scheduling: 4 B1 phases (q-half x head-pair) of scores+exp with V-projection
jobs, late projection groups and the previous phase's AV/epilogue/Wo work
interleaved into per-key-block PE slots; input DMAs are split so the first
exp fires as soon as the minimal K/Q bytes have landed.
V stays bf16 end-to-end (vh feeds the output directly); the fp8 paths only
perturb softmax weights, which the output is insensitive to.
"""

import functools

import numpy as np
import ml_dtypes

import concourse.bass as bass
import concourse.mybir as mybir
import concourse.tile as tile
from concourse import bacc
from concourse.bass_utils import run_bass_kernel_spmd
from concourse.masks import make_identity

BF16 = mybir.dt.bfloat16
F32 = mybir.dt.float32
F8 = mybir.dt.float8e4          # e4m3: Q/K inputs + weights (range +-240)
F8AV = mybir.dt.float8e5        # e5m2: exp'd scores + V for AV (wide range)
NP_F8 = ml_dtypes.float8_e4m3
NP_BF16 = ml_dtypes.bfloat16

B, S, D, H = 2, 2048, 1024, 16
DK = D // H          # 64
HG = 4               # heads per core (head group)
GD = HG * DK         # 256 head-group dims per core
NKC = D // 128       # 8 contraction chunks for projections
NKB = S // 128       # 16 key blocks
EXPB = -2.0          # exp bias; softmax-invariant, keeps e^x in fp8 range

DR = mybir.MatmulPerfMode.DoubleRow
Exp = mybir.ActivationFunctionType.Exp
Copy = mybir.ActivationFunctionType.Copy
MUL = mybir.AluOpType.mult
SUB = mybir.AluOpType.subtract
EQ = mybir.AluOpType.is_equal
NE = mybir.AluOpType.not_equal


def _emit_kernel(tc: tile.TileContext):
    nc = tc.nc

    # DRAM params. Host pre-reshapes so each input is one large DMA with
    # 2KB+ per-partition contiguous runs.
    qT8 = nc.declare_dram_parameter("qT8", [128, NKC, S], F8, isOutput=False).ap()
    kT8 = nc.declare_dram_parameter("kT8", [128, NKC, S], F8, isOutput=False).ap()
    vT = nc.declare_dram_parameter("vT", [128, NKC, S], BF16, isOutput=False).ap()
    wq8 = nc.declare_dram_parameter("wq8", [128, 2, NKC, 128], F8, isOutput=False).ap()
    wk8 = nc.declare_dram_parameter("wk8", [128, 2, NKC, 128], F8, isOutput=False).ap()
    wv = nc.declare_dram_parameter("wv", [128, NKC, GD], BF16, isOutput=False).ap()
    wo = nc.declare_dram_parameter("wo", [128, 2, D], BF16, isOutput=False).ap()
    rcol = nc.declare_dram_parameter("rcol", [128, NKB], F32, isOutput=False).ap()
    ccol = nc.declare_dram_parameter("ccol", [128, NKB], F32, isOutput=False).ap()
    y = nc.declare_dram_parameter("y", [S, D], BF16, isOutput=True).ap()

    # ---------------- resident SBUF ----------------
    consts = tc.alloc_tile_pool(name="consts", bufs=1)
    wq_sb = consts.tile([128, 2, NKC, 128], F8)
    wk_sb = consts.tile([128, 2, NKC, 128], F8)
    wv_sb = consts.tile([128, NKC, GD], BF16)
    wo_sb = consts.tile([128, 2, D], BF16)
    rr = consts.tile([128, NKB], F32)
    cc = consts.tile([128, NKB], F32)
    ones1 = consts.tile([128, 1], F8AV)
    ident = consts.tile([128, 128], BF16)
    actwarm = consts.tile([1, 2], F32)
    expbias = consts.tile([128, 1], F32)

    res = tc.alloc_tile_pool(name="res", bufs=1)
    # projected Q/K, bf16, head-pair layout: partitions = [head 2p | head 2p+1]
    qhT2 = [res.tile([128, S], BF16, name=f"qhT2_{p}") for p in range(2)]
    khT2 = [res.tile([128, S], BF16, name=f"khT2_{p}") for p in range(2)]
    vhc = res.tile([128, NKB, GD], BF16)     # vh * contrib, natural [s, d]
    vnat = res.tile([128, NKB, 264], F8AV)     # reaches-scaled V + ones cols
    catT = res.tile([128, 2, S], BF16)       # [d-half, q] transposed output
    catn = res.tile([128, 8, GD], BF16)      # negated epilogue out, natural
    # et[par]: exp'd scores for one phase (2 heads x 16 kb x 1024 q), fp8
    et = [res.tile([128, 2, NKB, 1024], F8AV, name=f"et_{p}") for p in range(2)]
    qT_sb = res.tile([128, NKC, S], F8)
    kT_sb = res.tile([128, NKC, S], F8)
    vT_sb = res.tile([128, NKC, S], BF16)
    consts.seal()
    res.seal()

    # constants
    nc.gpsimd.memset(ones1, 1.0)
    nc.gpsimd.memset(actwarm, 0.0)
    nc.gpsimd.memset(expbias, EXPB)
    make_identity(nc, ident)
    for h in range(HG):
        nc.gpsimd.memset(vnat[:, :, h * 66 + 64:h * 66 + 65], 1.0)
        nc.gpsimd.memset(vnat[:, :, h * 66 + 65:h * 66 + 66], 0.0)

    # input DMAs split and ordered so the first scores+exp can fire as early
    # as possible: the kT quarter covering key blocks 0-3, pair-0 weights,
    # the two qT quarters feeding q-half 0, then the rest in need order.
    def dmas(dst, src, lo, hi):
        nc.sync.dma_start(out=dst[:, :, lo:hi], in_=src[:, :, lo:hi])

    dmas(kT_sb, kT8, 0, 512)
    nc.sync.dma_start(out=wk_sb[:, 0], in_=wk8[:, 0])
    nc.sync.dma_start(out=wq_sb[:, 0], in_=wq8[:, 0])
    dmas(qT_sb, qT8, 0, 512)
    dmas(qT_sb, qT8, 512, 1024)
    nc.sync.dma_start(out=wk_sb[:, 1], in_=wk8[:, 1])
    nc.sync.dma_start(out=wq_sb[:, 1], in_=wq8[:, 1])
    dmas(kT_sb, kT8, 512, 1024)
    dmas(kT_sb, kT8, 1024, 2048)
    nc.sync.dma_start(out=wv_sb, in_=wv)
    nc.sync.dma_start(out=rr, in_=rcol)
    nc.sync.dma_start(out=cc, in_=ccol)
    dmas(vT_sb, vT, 0, 1024)
    dmas(vT_sb, vT, 1024, 2048)
    dmas(qT_sb, qT8, 1024, 2048)
    nc.sync.dma_start(out=wo_sb, in_=wo)

    with (
        tc.tile_pool(name="sc", bufs=2, space="PSUM") as sc,
        tc.tile_pool(name="acc", bufs=2, space="PSUM") as acc,
        tc.tile_pool(name="two", bufs=1, space="PSUM") as two,
        tc.tile_pool(name="dpool", bufs=34) as dpool,
        tc.tile_pool(name="spool", bufs=8) as spool,
        tc.tile_pool(name="ypool", bufs=6) as ypool,
    ):
        # Preload the exp activation table off the critical path.
        wrm = spool.tile([1, 2], F32, tag="wrm")
        nc.scalar.activation(wrm, actwarm, Exp)

        # Warm the PE p-state while input DMAs are in flight: ~4us of dummy
        # transposes (no DMA deps) so the first projections run at full rate.
        for _ in range(5):
            wtp = acc.tile([128, 512], F32, tag="acc", name="wtp")
            for j in range(4):
                nc.tensor.transpose(
                    wtp[:, j * 128:(j + 1) * 128].bitcast(BF16)[:, 0:128],
                    ident, ident)

        # ---- Q/K projections: fp8 DoubleRow into bf16 head-pair tiles ----
        def proj_grp(w_sb, x_sb, dst, p, lo, hi):
            ps = acc.tile([128, 512], F32, tag="acc", name="ps")
            for j in range(4):
                nc.tensor.matmul(
                    ps[:, 0:hi - lo],
                    lhsT=w_sb[:, p, 2 * j:2 * j + 2, :],
                    rhs=x_sb[:, 2 * j:2 * j + 2, lo:hi],
                    start=(j == 0), stop=(j == 3),
                    perf_mode=DR,
                )
            nc.vector.tensor_copy(dst[p][:, lo:hi], ps[:, 0:hi - lo])

        def kgrp(p, nq):
            proj_grp(wk_sb, kT_sb, khT2, p, nq * 512, (nq + 1) * 512)

        def qgrp(p, nq):
            proj_grp(wq_sb, qT_sb, qhT2, p, nq * 512, (nq + 1) * 512)

        # critical lead-in: only what B1 phase 0's first key blocks and
        # q-half-0 need, in DMA-arrival order
        kgrp(0, 0)
        qgrp(0, 0)
        qgrp(0, 1)

        # ---- V projection job for one 128-row block of S ----
        def vjob(ms):
            ps = acc.tile([128, 256], F32, tag="acc", name="ps")
            for kc in range(NKC):
                nc.tensor.matmul(
                    ps,
                    lhsT=vT_sb[:, kc, ms * 128:(ms + 1) * 128],
                    rhs=wv_sb[:, kc, :],
                    start=(kc == 0), stop=(kc == NKC - 1),
                )
            nc.vector.tensor_scalar_mul(vhc[:, ms, :], ps, cc[:, ms:ms + 1])
            nc.vector.tensor_scalar_mul(
                vnat[:, ms, :].rearrange("p (h c) -> p h c", h=4)[:, :, 0:64],
                ps.rearrange("p (h c) -> p h c", h=4),
                rr[:, ms:ms + 1])

        d2map = {}

        # ---- B2 work for one (consumed phase, global q-block) ----
        def b2qb(ph, qb, tail=False):
            half, hp = divmod(ph, 2)
            par = hp
            qoff = (qb % 8) * 128
            for hl in range(2):
                h = hp * 2 + hl
                av = acc.tile([128, 256], F32, tag="acc", name="av")
                for t in range(8):
                    nc.tensor.matmul(
                        av[:, 0:65],
                        lhsT=et[par][:, hl, 2 * t:2 * t + 2, qoff:qoff + 128],
                        rhs=vnat[:, 2 * t:2 * t + 2, h * 66:h * 66 + 65],
                        start=(t == 0), stop=(t == 7),
                        perf_mode=DR,
                    )
                    if t == 0:
                        # diag add-back: denominator must see the unmasked sum
                        nc.tensor.matmul(
                            av[:, 64:65],
                            lhsT=d2map[(ph, hl, qb)],
                            rhs=ones1,
                            start=False, stop=False,
                            skip_group_check=True,
                        )
                rec = spool.tile([128, 1], F32, tag="rec")
                nc.vector.reciprocal(rec, av[:, 64:65])
                s1 = spool.tile([128, 1], F32, tag="s1")
                nc.vector.tensor_mul(s1, rec, cc[:, qb:qb + 1])
                nc.vector.scalar_tensor_tensor(
                    out=catn[:, qb % 8, h * 64:(h + 1) * 64],
                    in0=av[:, 0:64],
                    scalar=s1,
                    in1=vhc[:, qb, h * 64:(h + 1) * 64],
                    op0=MUL, op1=SUB,
                )
            if hp == 1:
                if tail:
                    # Body-Wo's "two" slot (2 banks) is idle at tail: one
                    # transpose per bank, then a single merged DVE copy.
                    tp = two.tile([128, 2, 1024], BF16, tag="two", name="tp")
                    for dh in range(2):
                        nc.tensor.transpose(
                            tp[:, dh, 0:128],
                            catn[:, qb % 8, dh * 128:(dh + 1) * 128], ident)
                    nc.vector.tensor_copy(
                        catT[:, :, qb * 128:(qb + 1) * 128], tp[:, :, 0:128])
                else:
                    for dh in range(2):
                        tp = acc.tile([128, 128], BF16, tag="acc", name="tp")
                        nc.tensor.transpose(
                            tp, catn[:, qb % 8, dh * 128:(dh + 1) * 128], ident)
                        nc.vector.tensor_copy(
                            catT[:, dh, qb * 128:(qb + 1) * 128], tp)
                # At tail the scores pool is idle; borrowing its (same-sized)
                # slots gives Wo a 2-deep ring without extra PSUM banks.
                pool = sc if tail else two
                wop = pool.tile([128, 2, 512], F32, tag="sc" if tail else "two",
                                name="wop")
                for oc in range(2):
                    for dh in range(2):
                        nc.tensor.matmul(
                            wop[:, oc, :],
                            lhsT=catT[:, dh, qb * 128:(qb + 1) * 128],
                            rhs=wo_sb[:, dh, oc * 512:(oc + 1) * 512],
                            start=(dh == 0), stop=(dh == 1),
                        )
                ysb = ypool.tile([128, 1024], BF16, tag="ysb")
                if tail:
                    # per-oc copy + DMA: the copy starts right after its own
                    # oc's matmuls, and the last drain unit is smaller
                    for oc in range(2):
                        eng = nc.scalar if (qb % 2 == oc) else None
                        sl = ysb[:, oc * 512:(oc + 1) * 512]
                        if qb >= 14 and oc == 1:
                            nc.vector.tensor_copy(sl, wop[:, oc, :])
                        else:
                            nc.scalar.activation(sl, wop[:, oc, :], Copy)
                        nc.sync.dma_start(
                            out=y[qb * 128:(qb + 1) * 128,
                                  oc * 512:(oc + 1) * 512],
                            in_=sl)
                else:
                    nc.vector.tensor_copy(ysb, wop)
                    nc.sync.dma_start(
                        out=y[qb * 128:(qb + 1) * 128, :], in_=ysb)

        # ---- B1 phase: scores + exp (+ diag) with interleaved fill jobs ----
        # Per-phase PE fill schedules, keyed by kb slot. Late projection
        # groups and V jobs ride in phase 0/1 slots (ordered so their DMA
        # halves have landed); B2 consumption starts in phase 1's second
        # half (vnat complete by then) and runs on odd slots afterwards.
        def sched_jobs(i, kb):
            jobs = []
            if i == 0:
                if kb == 0:
                    jobs.append(lambda: kgrp(0, 1))
                elif kb == 1:
                    jobs.append(lambda: kgrp(0, 2))
                elif kb == 2:
                    jobs.append(lambda: kgrp(0, 3))
                elif kb == 4:
                    jobs += [lambda: kgrp(1, 0), lambda: kgrp(1, 1)]
                elif kb == 5:
                    jobs += [lambda: kgrp(1, 2), lambda: kgrp(1, 3)]
                elif kb == 6:
                    jobs += [lambda: qgrp(1, 0), lambda: qgrp(1, 1)]
                elif kb >= 8:
                    ms = kb - 8
                    jobs.append(lambda: vjob(ms))
            elif i == 1:
                if kb < 8:
                    ms = kb + 8
                    jobs.append(lambda: vjob(ms))
                else:
                    jobs.append(lambda: b2qb(0, kb - 8))
                    if kb % 2 == 0:
                        # late q projections ride the PE-light B2 slots
                        pj, pn = divmod((kb - 8) // 2, 2)
                        jobs.append(lambda: qgrp(pj, 2 + pn))
            else:
                if kb % 2 == 1:
                    prev = i - 1
                    qb = 8 * (prev // 2) + kb // 2
                    jobs.append(lambda: b2qb(prev, qb))
            return jobs

        def b1phase(i):
            half, hp = divmod(i, 2)
            par = hp
            q0 = half * 1024
            if i == 0:
                # Bootstrap kb0 at qc granularity: both heads' first-half
                # exps only need the first projected q chunk, so they fill
                # the wait for the second chunk's DMA+projection.
                sps = [sc.tile([128, 1024], F32, tag="sc", name="sp")
                       for _ in range(2)]
                for qc in range(2):
                    for hl in range(2):
                        r0, r1 = hl * 64, hl * 64 + 64
                        nc.tensor.matmul(
                            sps[hl][:, qc * 512:(qc + 1) * 512],
                            lhsT=khT2[0][r0:r1, 0:128],
                            rhs=qhT2[0][r0:r1, qc * 512:(qc + 1) * 512],
                            start=True, stop=True,
                        )
                        nc.scalar.activation(
                            et[0][:, hl, 0, qc * 512:(qc + 1) * 512],
                            sps[hl][:, qc * 512:(qc + 1) * 512],
                            Exp, scale=0.125, bias=expbias)
                    if qc == 0:
                        for hl in range(2):
                            esl = et[0][:, hl, 0, 0:128]
                            d2t = dpool.tile([128, 128], F8AV, tag="d2",
                                             name="d2t")
                            nc.gpsimd.affine_select(
                                out=d2t, in_=esl, compare_op=EQ,
                                fill=0.0, base=0, pattern=[[-1, 128]],
                                channel_multiplier=1)
                            nc.gpsimd.affine_select(
                                out=esl, in_=esl, compare_op=NE,
                                fill=0.0, base=0, pattern=[[-1, 128]],
                                channel_multiplier=1)
                            d2map[(0, hl, 0)] = d2t
            for kb in range(NKB):
                for hl in range(2):
                    if i == 0 and kb == 0:
                        continue
                    r0, r1 = hl * 64, hl * 64 + 64
                    sp = sc.tile([128, 1024], F32, tag="sc", name="sp")
                    for qc in range(2):
                        nc.tensor.matmul(
                            sp[:, qc * 512:(qc + 1) * 512],
                            lhsT=khT2[hp][r0:r1, kb * 128:(kb + 1) * 128],
                            rhs=qhT2[hp][r0:r1, q0 + qc * 512:q0 + (qc + 1) * 512],
                            start=True, stop=True,
                        )
                    nc.scalar.activation(
                        et[par][:, hl, kb, :], sp, Exp, scale=0.125,
                        bias=expbias)
                    if 8 * half <= kb < 8 * half + 8:
                        off = (kb - 8 * half) * 128
                        esl = et[par][:, hl, kb, off:off + 128]
                        d2t = dpool.tile([128, 128], F8AV, tag="d2", name="d2t")
                        nc.gpsimd.affine_select(
                            out=d2t, in_=esl, compare_op=EQ,
                            fill=0.0, base=0, pattern=[[-1, 128]],
                            channel_multiplier=1)
                        nc.gpsimd.affine_select(
                            out=esl, in_=esl, compare_op=NE,
                            fill=0.0, base=0, pattern=[[-1, 128]],
                            channel_multiplier=1)
                        d2map[(i, hl, kb)] = d2t
                for job in sched_jobs(i, kb):
                    job()

        for i in range(4):
            b1phase(i)
        for qb in range(8, 16):
            b2qb(3, qb, tail=True)


@functools.cache
def build_nc() -> bass.Bass:
    nc = bacc.Bacc("TRN2", target_bir_lowering=False, debug=False)
    with tile.TileContext(nc) as tc:
        _emit_kernel(tc)
    nc.compile()
    return nc


def _to_chunked(x, dtype, chunk=128):
    """[R, C] -> [chunk, R//chunk, C] so partition p's data is contiguous."""
    r, c = x.shape
    return np.ascontiguousarray(
        x.reshape(r // chunk, chunk, c).transpose(1, 0, 2)).astype(dtype)


def _to_pairmajor(w, dtype):
    """[D, 256] -> [128, 2, 8, 128]: per-partition pair-major contiguous."""
    c = _to_chunked(w, dtype)              # [128, 8, 256]
    return np.ascontiguousarray(
        c.reshape(128, 8, 2, 128).transpose(0, 2, 1, 3))


def _prep_inputs(q, k, v, reaches, Wq, Wk, Wv, Wo):
    """Host-side shard + layout prep. Returns per-core input maps."""
    r = np.asarray(reaches, np.float32)
    rs = r.sum(axis=-1, keepdims=True)
    contrib = (rs - r) / (rs + 1e-9) * (1.0 - r) * 100.0  # [B, S] f32

    per_batch = []
    for b in range(B):
        qTb = _to_chunked(np.asarray(q[b], np.float32).T, NP_F8)
        kTb = _to_chunked(np.asarray(k[b], np.float32).T, NP_F8)
        vTb = _to_chunked(np.asarray(v[b], np.float32).T, NP_BF16)
        rcol = np.ascontiguousarray(r[b].reshape(NKB, 128).T)
        ccol = np.ascontiguousarray(contrib[b].reshape(NKB, 128).T)
        per_batch.append((qTb, kTb, vTb, rcol, ccol))

    in_maps = []
    for c in range(8):
        b, g = divmod(c, 4)
        hs = slice(g * GD, (g + 1) * GD)
        qTb, kTb, vTb, rcol, ccol = per_batch[b]
        wqh = np.asarray(Wq, np.float32)[hs, :].T
        wkh = np.asarray(Wk, np.float32)[hs, :].T
        wvh = np.asarray(Wv, np.float32)[hs, :].T
        woh = -np.asarray(Wo, np.float32)[:, hs].T
        in_maps.append({
            "qT8": qTb, "kT8": kTb, "vT": vTb,
            "wq8": _to_pairmajor(wqh, NP_F8),
            "wk8": _to_pairmajor(wkh, NP_F8),
            "wv": _to_chunked(wvh, NP_BF16),
            "wo": _to_chunked(woh, NP_BF16),
            "rcol": rcol, "ccol": ccol,
        })
    return in_maps


def kernel(q, k, v, reaches, Wq, Wk, Wv, Wo, **run_kwargs):
    nc = build_nc()
    in_maps = _prep_inputs(q, k, v, reaches, Wq, Wk, Wv, Wo)
    res = run_bass_kernel_spmd(nc, in_maps, list(range(8)), **run_kwargs)
    out = np.zeros((B, S, D), np.float32)
    for c in range(8):
        b = c // 4
        out[b] += np.asarray(res.results[c]["y"], np.float32)
    if run_kwargs:
        kernel.last_results = res
    return out
